# revision 1
# baseline (speedup 1.0000x reference)
"""Multi-head attention (softmax-over-query-axis variant) on 8 TRN2 NeuronCores.

Sharding: core c handles batch b = c // 2 and head group g = c % 2
(heads [8g, 8g+8)).  Each core computes its heads' context and a
row-sharded partial of the output projection; the host sums the two
partials per batch and adds the bias.

Reference semantics (B=4, T=2048, E=1024, H=16, HS=64):
  Q = einsum('bte,hed->bhtd', x, Wq); same K, V
  S = Q K^T / sqrt(E)   (sqrt(E), not sqrt(HS))
  causal mask; softmax over the QUERY axis (axis=2 of (B,H,Tq,Tk))
  out = (softmax(S) V) reshaped -> (B,T,E); out @ Wo + bo

On-device algorithm per (head, key-block s of 128):
  S^T[s, t] tiles from PE (keys on partitions); causal mask applied with
  tensor_mask_reduce (-FLT_MAX fill) on PSUM; exp + row-sum in one ACT
  instruction (accum_out); Z[s] = sum over t >= s of exp(S^T); the 1/Z
  normalization is folded into V (V'[s,:] = V[s,:]/Z[s]) so attn@V is a
  plain matmul accumulation over key blocks.
"""

import os
import sys

for _p in ("/opt/trn_rl_repo", "/root/.axon_site/_ro/trn_rl_repo"):
    if os.path.isdir(_p) and _p not in sys.path:
        sys.path.insert(0, _p)

import numpy as np

import concourse.bass as bass
import concourse.mybir as mybir
import concourse.tile as tile
from concourse import bacc
from concourse.masks import make_identity

F32 = mybir.dt.float32
F32R = mybir.dt.float32r
BF16 = mybir.dt.bfloat16
ALU = mybir.AluOpType
ACT = mybir.ActivationFunctionType

B, T, E, H, HS = 4, 2048, 1024, 16, 64
N_CORES = 8
HEADS_PER_CORE = H // 2        # 8
N_PAIRS = HEADS_PER_CORE // 2  # 4
SCALE = 1.0 / float(np.sqrt(np.float32(E)))  # 1/32
NEG_MASK = -1e30  # exp(NEG_MASK * SCALE) == 0.0 in fp32
NB = T // 128                  # 16 key/query blocks of 128
NT = T // 512                  # 4 t tiles of 512


def build_kernel(repeat: int = 1) -> bass.Bass:
    """Build the per-core SPMD kernel.  `repeat` emits the body N times
    (for wall-clock timing via deltas); outputs are simply overwritten."""
    nc = bacc.Bacc("TRN2", target_bir_lowering=False, debug=False,
                   enable_asserts=True, num_devices=N_CORES)

    xb = nc.dram_tensor("xb", [T, E], F32, kind="ExternalInput").ap()
    wq = nc.dram_tensor("wq", [E, 512], F32R, kind="ExternalInput").ap()
    wk = nc.dram_tensor("wk", [E, 512], F32R, kind="ExternalInput").ap()
    wv = nc.dram_tensor("wv", [E, 512], F32R, kind="ExternalInput").ap()
    wo = nc.dram_tensor("wo", [512, E], F32R, kind="ExternalInput").ap()
    outp = nc.dram_tensor("outp", [T, E], F32, kind="ExternalOutput").ap()
    ct_dram = nc.dram_tensor("ct_scratch", [512, T], F32R).ap()

    with tile.TileContext(nc) as tc:
        with (
            tc.tile_pool(name="const", bufs=1) as cpool,
            tc.tile_pool(name="sb", bufs=1) as sb,
            tc.tile_pool(name="ps", bufs=1, space="PSUM") as ps,
        ):
            # ---- constants ----
            ident = cpool.tile([128, 128], F32)
            make_identity(nc, ident)
            # tri_below[p, c] = 1 where c < p (strictly below diagonal), else 0
            tri_below = cpool.tile([128, 128], mybir.dt.int32)
            nc.gpsimd.memset(tri_below, 1.0)
            nc.gpsimd.affine_select(
                out=tri_below, in_=tri_below, pattern=[[-1, 128]],
                compare_op=ALU.is_ge, fill=0.0, base=-1, channel_multiplier=1)
            neg_tile = cpool.tile([128, 128], F32)
            nc.gpsimd.memset(neg_tile, NEG_MASK)

            for _rep in range(repeat):
                body(nc, sb, ps, ident, tri_below, neg_tile,
                     xb, wq, wk, wv, wo, outp, ct_dram)

    nc.compile()
    return nc


def body(nc, sb, ps, ident, tri_below, neg_tile, xb, wq, wk, wv, wo, outp, ct_dram):
    # ---- phase 0: load x and transpose into xT (8 tiles of [128e, 2048t]) ----
    xts = []
    for e in range(8):
        xt_e = sb.tile([128, T], F32, name=f"xt{e}", tag=f"xt{e}", bufs=1)
        xts.append(xt_e)
    for t4 in range(4):
        stages = []
        for dt in range(4):
            st = sb.tile([128, E], F32, name=f"xstage_{t4}_{dt}", tag="xstage", bufs=4)
            nc.sync.dma_start(out=st, in_=xb[(4 * t4 + dt) * 128:(4 * t4 + dt + 1) * 128, :])
            stages.append(st)
        for e in range(8):
            pt = ps.tile([128, 512], F32, name=f"tp_{t4}_{e}", tag="mm", bufs=2)
            for dt in range(4):
                nc.tensor.transpose(pt[:, 128 * dt:128 * (dt + 1)],
                                    stages[dt][:, 128 * e:128 * (e + 1)], ident)
            nc.vector.tensor_copy(xts[e][:, 512 * t4:512 * (t4 + 1)].bitcast(F32R), pt)

    # ---- per head-pair: projections + attention ----
    ct_tiles_meta = []  # (pair) -> nothing kept in SBUF; context goes to ct_dram
    for p in range(N_PAIRS):
        # -- projections for local heads (2p, 2p+1) --
        qt = sb.tile([128, T], F32, name=f"qt_p{p}", tag="qt", bufs=2)
        kt = sb.tile([128, T], F32, name=f"kt_p{p}", tag="kt", bufs=2)
        vtmp = sb.tile([128, T], F32, name=f"vtmp_p{p}", tag="vtmp", bufs=1)
        for iproj, (wdram, dest) in enumerate(((wq, qt), (wk, kt), (wv, vtmp))):
            wt = sb.tile([128, E], F32R, name=f"w{iproj}_p{p}", tag=f"w{iproj}", bufs=1)
            nc.sync.dma_start(
                out=wt.rearrange("b (a c) -> b a c", a=8),
                in_=wdram[:, 128 * p:128 * (p + 1)].rearrange("(a b) c -> b a c", b=128))
            for tt in range(NT):
                pmm = ps.tile([128, 512], F32, name=f"proj_{p}_{iproj}_{tt}", tag="mm", bufs=2)
                for e in range(8):
                    nc.tensor.matmul(pmm, lhsT=wt[:, 128 * e:128 * (e + 1)].bitcast(F32R),
                                     rhs=xts[e][:, 512 * tt:512 * (tt + 1)].bitcast(F32R),
                                     start=(e == 0), stop=(e == 7))
                dslice = dest[:, 512 * tt:512 * (tt + 1)]
                if dest is not vtmp:
                    dslice = dslice.bitcast(F32R)
                nc.vector.tensor_copy(dslice, pmm)

        # -- V transpose: vtmp [dpair 128, s 2048] -> v_p chunks [s 128, dpair 128] --
        v_p = sb.tile([128, T], F32, name=f"v_p{p}", tag="v", bufs=2)
        for s4 in range(4):
            pt = ps.tile([128, 512], F32, name=f"vt_{p}_{s4}", tag="mm", bufs=2)
            for ds_ in range(4):
                nc.tensor.transpose(pt[:, 128 * ds_:128 * (ds_ + 1)],
                                    vtmp[:, 128 * (4 * s4 + ds_):128 * (4 * s4 + ds_ + 1)],
                                    ident)
            nc.vector.tensor_copy(v_p[:, 512 * s4:512 * (s4 + 1)], pt)

        # -- attention --
        o_tiles = [ps.tile([128, 512], F32, name=f"o_{p}_{j}", tag=f"o{j}", bufs=1)
                   for j in range(4)]
        for i in range(NB):
            j0 = i // 4
            v4 = i % 4
            c0 = 128 * v4
            ea_tiles, eb_tiles = {}, {}
            za_list, zb_list = [], []
            for j in range(j0, 4):
                psA = ps.tile([128, 512], F32, name=f"scA_{p}_{i}_{j}", tag="scA", bufs=1)
                psB = ps.tile([128, 512], F32, name=f"scB_{p}_{i}_{j}", tag="scB", bufs=1)
                nc.tensor.matmul(psA, lhsT=kt[0:64, 128 * i:128 * (i + 1)].bitcast(F32R),
                                 rhs=qt[0:64, 512 * j:512 * (j + 1)].bitcast(F32R),
                                 start=True, stop=True, tile_position=(0, 0))
                nc.tensor.matmul(psB, lhsT=kt[64:128, 128 * i:128 * (i + 1)].bitcast(F32R),
                                 rhs=qt[64:128, 512 * j:512 * (j + 1)].bitcast(F32R),
                                 start=True, stop=True, tile_position=(64, 0))
                if j == j0:
                    # causal mask: columns < c0 fully invalid; the 128-wide
                    # block at c0 is masked strictly-below-diagonal
                    for pX in (psA, psB):
                        if c0 > 0:
                            nc.vector.memset(pX[:, 0:c0], NEG_MASK)
                        nc.vector.copy_predicated(
                            out=pX[:, c0:c0 + 128], mask=tri_below, data=neg_tile)
                ea = sb.tile([128, 512], BF16, name=f"eA_{p}_{i}_{j}", tag="eA", bufs=5)
                eb = sb.tile([128, 512], BF16, name=f"eB_{p}_{i}_{j}", tag="eB", bufs=5)
                za = sb.tile([128, 1], F32, name=f"zA_{p}_{i}_{j}", tag="z", bufs=24)
                zb = sb.tile([128, 1], F32, name=f"zB_{p}_{i}_{j}", tag="z", bufs=24)
                nc.scalar.activation(ea, psA, ACT.Exp, bias=0.0, scale=SCALE, accum_out=za)
                nc.scalar.activation(eb, psB, ACT.Exp, bias=0.0, scale=SCALE, accum_out=zb)
                ea_tiles[j], eb_tiles[j] = ea, eb
                za_list.append(za)
                zb_list.append(zb)

            # Z and 1/Z per head, then V' = V / Z
            recips = []
            for zl in (za_list, zb_list):
                acc = zl[0]
                for v_ in zl[1:]:
                    nxt = sb.tile([128, 1], F32, name=f"zs_{p}_{i}", tag="z", bufs=24)
                    nc.vector.tensor_tensor(nxt, acc, v_, ALU.add)
                    acc = nxt
                rz = sb.tile([128, 1], F32, name=f"rz_{p}_{i}", tag="z", bufs=24)
                nc.vector.reciprocal(rz, acc)
                recips.append(rz)
            vp_t = sb.tile([128, 128], BF16, name=f"vp_{p}_{i}", tag="vp", bufs=3)
            nc.vector.tensor_scalar_mul(vp_t[:, 0:64],
                                        v_p[:, 128 * i:128 * i + 64], recips[0])
            nc.vector.tensor_scalar_mul(vp_t[:, 64:128],
                                        v_p[:, 128 * i + 64:128 * (i + 1)], recips[1])

            for j in range(j0, 4):
                nc.tensor.matmul(o_tiles[j][0:64, :], lhsT=vp_t[:, 0:64],
                                 rhs=ea_tiles[j],
                                 start=(i == 0), stop=(i == 4 * j + 3),
                                 tile_position=(0, 0), skip_group_check=True)
                nc.tensor.matmul(o_tiles[j][64:128, :], lhsT=vp_t[:, 64:128],
                                 rhs=eb_tiles[j],
                                 start=(i == 0), stop=(i == 4 * j + 3),
                                 tile_position=(0, 64), skip_group_check=True)
                if i == 4 * j + 3:
                    ostage = sb.tile([128, 512], F32R, name=f"oct_{p}_{j}", tag="oct", bufs=3)
                    nc.vector.tensor_copy(ostage, o_tiles[j])
                    nc.sync.dma_start(
                        out=ct_dram[128 * p:128 * (p + 1), 512 * j:512 * (j + 1)],
                        in_=ostage)

    # ---- output projection: out[t, :] = ct[:, t].T @ wo (row shard) ----
    ct_sb = []
    for ein in range(4):
        cl = sb.tile([128, T], F32R, name=f"ctl{ein}", tag=("qt" if ein < 2 else "kt"), bufs=2)
        nc.sync.dma_start(out=cl, in_=ct_dram[128 * ein:128 * (ein + 1), :])
        ct_sb.append(cl)
    wo_sb = []
    for ein in range(4):
        wt = sb.tile([128, E], F32R, name=f"wol{ein}", tag="xstage", bufs=4)
        nc.sync.dma_start(out=wt, in_=wo[128 * ein:128 * (ein + 1), :])
        wo_sb.append(wt)
    for tb in range(NB):
        for eo in range(2):
            pmm = ps.tile([128, 512], F32, name=f"op_{tb}_{eo}", tag="mm", bufs=2)
            for ein in range(4):
                nc.tensor.matmul(pmm, lhsT=ct_sb[ein][:, 128 * tb:128 * (tb + 1)].bitcast(F32R),
                                 rhs=wo_sb[ein][:, 512 * eo:512 * (eo + 1)].bitcast(F32R),
                                 start=(ein == 0), stop=(ein == 3))
            ostage = sb.tile([128, 512], F32, name=f"ost_{tb}_{eo}", tag="ost", bufs=3)
            nc.vector.tensor_copy(ostage, pmm)
            nc.sync.dma_start(out=outp[128 * tb:128 * (tb + 1), 512 * eo:512 * (eo + 1)],
                              in_=ostage)


def make_in_maps(x, Wq, Wk, Wv, Wo):
    """Shard full inputs into per-core input maps."""
    in_maps = []
    for c in range(N_CORES):
        b, g = c // 2, c % 2
        heads = range(8 * g, 8 * g + 8)
        in_maps.append({
            "xb": np.ascontiguousarray(x[b], dtype=np.float32),
            "wq": np.concatenate([Wq[h] for h in heads], axis=1).astype(np.float32),
            "wk": np.concatenate([Wk[h] for h in heads], axis=1).astype(np.float32),
            "wv": np.concatenate([Wv[h] for h in heads], axis=1).astype(np.float32),
            "wo": np.ascontiguousarray(Wo[512 * g:512 * (g + 1), :], dtype=np.float32),
        })
    return in_maps


_NC_CACHE = {}


def _get_nc(repeat: int = 1):
    if repeat not in _NC_CACHE:
        _NC_CACHE[repeat] = build_kernel(repeat)
    return _NC_CACHE[repeat]


def kernel(x, Wq, Wk, Wv, Wo, bo):
    from concourse.bass_utils import run_bass_kernel_spmd

    nc = _get_nc()
    in_maps = make_in_maps(np.asarray(x), np.asarray(Wq), np.asarray(Wk),
                           np.asarray(Wv), np.asarray(Wo))
    res = run_bass_kernel_spmd(nc, in_maps, core_ids=list(range(N_CORES)))
    bo = np.asarray(bo, dtype=np.float32)
    out = np.empty((B, T, E), dtype=np.float32)
    for b in range(B):
        out[b] = res.results[2 * b]["outp"] + res.results[2 * b + 1]["outp"] + bo
    return out

